# revision 19
# baseline (speedup 1.0000x reference)
"""Trainium2 Bass kernel for nn_LLMBinaryMultitaskMLPGenerator.

out[b,s,t] = sigmoid(relu(relu(relu(x) @ W1[t] + b1[t]) @ W2[t] + b2[t]) @ W3[t] + b3[t])

Sharding: task-parallel across 8 cores (2 tasks per core, all 8192 rows).

All three GEMMs run in fp8 (e4m3) with DoubleRow perf mode: operands are
packed [128, 2, ...] so each matmul contracts 256 deep at 2x the bf16
MAC rate, accumulating fp32 in PSUM. Host-side layout prep scales the
operands into e4m3's normal range (x*16, W*64, activations*8); PSUM
eviction folds the dequant scale + bias + relu/sigmoid into one
scalar-engine activation op (L1, L3) or a DVE tensor_scalar mult+max
(L2, valid because b2==0; a scalar-engine variant is built if any b2
is nonzero). relu(x) runs in place, split DVE / Pool engines.

Numerics (numpy sim vs fp32 reference): rel_l2 ~1e-2, gate is 2e-2.

Engine budget per 2048-col chunk (8 chunks/rep), HW estimates:
  PE ~18us, ACT ~12us, DVE ~8us, Pool ~5us, DMA ~6us in + tiny out.

Layouts (per core, TL=2 tasks):
  xq  [TL, 128, 8, N] fp8   = e4m3(16*x), d = db*128+p (partition-major)
  w1  [TL, 4, 128, 2, 512]  = e4m3(64*W1), d = kb*256 + i*128 + p
  b1  [TL, 4, 128, 1] f32   = 8*b1
  w2  [TL, 2, 128, 2, 256]  = e4m3(64*W2), h = pb*256 + i*128 + p
  b2  [TL, 2, 128, 1] f32   = 8*b2
  w3  [TL, 128, 2, 1] fp8   = e4m3(64*W3), k = i*128 + p
  b3  [TL, 1, 1] f32
  out [TL, 1, N] f32
"""

import sys

sys.path.insert(0, "/opt/trn_rl_repo")

from contextlib import ExitStack

import numpy as np

import concourse.bass as bass  # noqa: F401  (engine namespaces live on nc)
import concourse.mybir as mybir
import concourse.tile as tile
from concourse import bacc
from concourse.bass_utils import run_bass_kernel_spmd

import jax

jax.config.update("jax_compilation_cache_dir", "/tmp/jaxcache")
jax.config.update("jax_persistent_cache_min_compile_time_secs", 0.0)
jax.config.update("jax_persistent_cache_min_entry_size_bytes", -1)

F32 = mybir.dt.float32
FP8 = mybir.dt.float8e4
AFT = mybir.ActivationFunctionType
ALU = mybir.AluOpType
DR = mybir.MatmulPerfMode.DoubleRow

NCORES = 8
B, S, T, D, H1, H2 = 4, 2048, 16, 1024, 512, 256
N = B * S  # 8192 rows per task, replicated on every core
TL = T // NCORES  # 2 tasks per core
NDB = D // 128  # 8 x d-blocks
NKB1 = D // 256  # 4 L1 contraction pair-blocks
NHB = H1 // 128  # 4 h1 feature blocks
NPB2 = H1 // 256  # 2 L2 contraction pair-blocks
NKB2 = H2 // 128  # 2 h2 feature blocks
IC_CHUNK = 2048  # n-columns per pipeline chunk
SC = 512  # matmul moving free dim / PSUM bank width (fp32 max)
NIC = N // IC_CHUNK
NSC = IC_CHUNK // SC
NDVE = 5  # x-relu d-blocks on DVE; the rest go to Pool (gpsimd)
EVICT_L1_DVE = 3  # L1 psum groups per chunk evicted on DVE when biases are 0

XS, WS, HS = 16.0, 64.0, 8.0  # host-side quant scales for x, weights, activations

TRACE = False
LAST_RESULT = None  # BassKernelResults of the last kernel() call


def _build_program(reps: int = 1, b2_nonzero: bool = False):
    nc = bacc.Bacc("TRN2", target_bir_lowering=False, debug=False, num_devices=NCORES)

    xq = nc.dram_tensor("xq", [TL, 128, NDB, N], FP8, kind="ExternalInput").ap()
    w1 = nc.dram_tensor("w1", [TL, 128, NKB1, 2, H1], FP8, kind="ExternalInput").ap()
    b1 = nc.dram_tensor("b1", [TL, 128, NHB], F32, kind="ExternalInput").ap()
    w2 = nc.dram_tensor("w2", [TL, 128, NPB2, 2, H2], FP8, kind="ExternalInput").ap()
    b2 = nc.dram_tensor("b2", [TL, 128, NKB2], F32, kind="ExternalInput").ap()
    w3 = nc.dram_tensor("w3", [TL, 128, 2, 1], FP8, kind="ExternalInput").ap()
    b3 = nc.dram_tensor("b3", [TL, 1, 1], F32, kind="ExternalInput").ap()
    out = nc.dram_tensor("out", [TL, 1, N], F32, kind="ExternalOutput").ap()

    with tile.TileContext(nc) as tc, ExitStack() as ctx:
        wpool = ctx.enter_context(tc.tile_pool(name="w", bufs=1))
        xpool = ctx.enter_context(tc.tile_pool(name="x", bufs=3))
        h1pool = ctx.enter_context(tc.tile_pool(name="h1", bufs=4))
        h2pool = ctx.enter_context(tc.tile_pool(name="h2", bufs=2))
        opool = ctx.enter_context(tc.tile_pool(name="o", bufs=8))
        l1ps = ctx.enter_context(tc.tile_pool(name="l1ps", bufs=4, space="PSUM"))
        l2ps = ctx.enter_context(tc.tile_pool(name="l2ps", bufs=2, space="PSUM"))
        l3ps = ctx.enter_context(tc.tile_pool(name="l3ps", bufs=2, space="PSUM"))

        # --- persistent per-task weights/biases in SBUF (one DMA each) ---
        w1s, w2s, w3s, b1s, b2s, b3s = [], [], [], [], [], []
        for t in range(TL):
            w1s.append(wpool.tile([128, NKB1, 2, H1], FP8, tag=f"w1_{t}", name=f"w1t_{t}"))
            w2s.append(wpool.tile([128, NPB2, 2, H2], FP8, tag=f"w2_{t}", name=f"w2t_{t}"))
            # pair-dim stride must be even and 16B-aligned for dual-fp8
            # LdWeights (walrus s3_lw_dual_fp8_restrictions), so pad to 16
            w3s.append(wpool.tile([128, 2, 16], FP8, tag=f"w3_{t}", name=f"w3t_{t}"))
            b1s.append(wpool.tile([128, NHB], F32, tag=f"b1_{t}", name=f"b1t_{t}"))
            b2s.append(wpool.tile([128, NKB2], F32, tag=f"b2_{t}", name=f"b2t_{t}"))
            b3s.append(wpool.tile([1, 1], F32, tag=f"b3_{t}", name=f"b3t_{t}"))

        def _load_weights(t):
            nc.sync.dma_start(w1s[t][:], w1[t])
            nc.sync.dma_start(b1s[t][:], b1[t])
            nc.sync.dma_start(w2s[t][:], w2[t])
            nc.sync.dma_start(b2s[t][:], b2[t])
            nc.sync.dma_start(w3s[t][:, :, 0:1], w3[t])
            nc.sync.dma_start(b3s[t][:], b3[t])

        def _body(load_weights):
            _pipeline(nc, tc, xq, out, w1s, w2s, w3s, b1s, b2s, b3s,
                      xpool, h1pool, h2pool, opool, l1ps, l2ps, l3ps, b2_nonzero,
                      load_weights)

        if reps == 1:
            # weight DMAs are interleaved after the first x-chunk prefetches
            # so the first compute chunk starts as early as possible
            _body(_load_weights)
        else:
            for t in range(TL):
                _load_weights(t)
            with tc.For_i(0, reps, 1):
                _body(None)

    nc.compile()
    return nc


def _pipeline(nc, tc, xq, out, w1s, w2s, w3s, b1s, b2s, b3s,
              xpool, h1pool, h2pool, opool, l1ps, l2ps, l3ps, b2_nonzero,
              load_weights=None):
    chunks = [(t, ic) for t in range(TL) for ic in range(NIC)]
    xtiles = {}

    def _prep(ci):
        # x: 4 batched DMAs per chunk, relu'd in place on DVE per kb-pair so
        # L1's kb-ordered matmuls can start as soon as pair 0 is ready.
        # Issued 2 chunks ahead of compute so the relu sits ahead of the
        # previous chunk's L2 evictions in the DVE queue.
        t, ic = chunks[ci]
        n0 = ic * IC_CHUNK
        xs = xpool.tile([128, NDB, IC_CHUNK], FP8, tag="x", name=f"x_{t}_{ic}")
        for kb in range(NKB1):
            pair = xs[:, 2 * kb : 2 * kb + 2, :]
            nc.sync.dma_start(
                pair, xq[t, :, 2 * kb : 2 * kb + 2, n0 : n0 + IC_CHUNK]
            )
        # relu per d-block: first NDVE on DVE, rest on Pool (gpsimd); Pool
        # can't touch PSUM so it can't help with evictions, only here
        for db in range(NDB):
            blk = xs[:, db : db + 1, :]
            eng = nc.vector if db < NDVE else nc.gpsimd
            eng.tensor_scalar_max(blk, blk, 0.0)
        xtiles[ci] = xs

    _prep(0)
    if load_weights is not None:
        load_weights(0)
    _prep(1)
    if load_weights is not None:
        for t in range(1, TL):
            load_weights(t)
    for ci, (t, ic) in enumerate(chunks):
            n0 = ic * IC_CHUNK
            xs = xtiles.pop(ci)
            if ci + 2 < len(chunks):
                _prep(ci + 2)

            # L1: h1 = 8*relu(z1 + b1), fp8, pair-packed over hb (scalar engine)
            h1t = [h1pool.tile([128, 2, IC_CHUNK], FP8, tag="h1", name=f"h1_{t}_{ic}_{pb}")
                   for pb in range(NPB2)]
            for hb in range(NHB):
                for sc in range(NSC):
                    ps = l1ps.tile([128, SC], F32, tag="l1", name=f"l1ps_{t}_{ic}_{hb}_{sc}")
                    for kb in range(NKB1):
                        nc.tensor.matmul(
                            ps[:],
                            w1s[t][:, kb, :, hb * 128 : (hb + 1) * 128],
                            xs[:, 2 * kb : 2 * kb + 2, sc * SC : (sc + 1) * SC],
                            start=(kb == 0),
                            stop=(kb == NKB1 - 1),
                            perf_mode=DR,
                        )
                    h1view = h1t[hb // 2][:, hb % 2, sc * SC : (sc + 1) * SC]
                    gidx = hb * NSC + sc
                    if not b2_nonzero and gidx >= NHB * NSC - EVICT_L1_DVE:
                        nc.vector.tensor_scalar(
                            h1view, ps[:], HS / (XS * WS), 0.0,
                            op0=ALU.mult, op1=ALU.max,
                        )
                    else:
                        nc.scalar.activation(
                            h1view, ps[:], AFT.Relu,
                            bias=b1s[t][:, hb : hb + 1],
                            scale=HS / (XS * WS),
                        )

            # L2: h2 = 8*relu(z2 + b2), fp8, pair-packed over kb (DVE eviction)
            h2t = h2pool.tile([128, 2, IC_CHUNK], FP8, tag="h2", name=f"h2_{t}_{ic}")
            for kb in range(NKB2):
                for sc in range(NSC):
                    ps = l2ps.tile([128, SC], F32, tag="l2", name=f"l2ps_{t}_{ic}_{kb}_{sc}")
                    for pb in range(NPB2):
                        nc.tensor.matmul(
                            ps[:],
                            w2s[t][:, pb, :, kb * 128 : (kb + 1) * 128],
                            h1t[pb][:, :, sc * SC : (sc + 1) * SC],
                            start=(pb == 0),
                            stop=(pb == NPB2 - 1),
                            perf_mode=DR,
                        )
                    h2view = h2t[:, kb, sc * SC : (sc + 1) * SC]
                    if b2_nonzero:
                        nc.scalar.activation(
                            h2view, ps[:], AFT.Relu,
                            bias=b2s[t][:, kb : kb + 1], scale=HS / (HS * WS),
                        )
                    else:
                        nc.vector.tensor_scalar(
                            h2view, ps[:], HS / (HS * WS), 0.0,
                            op0=ALU.mult, op1=ALU.max,
                        )

            # L3: out = sigmoid(z3 + b3); out DMA issued by the scalar engine
            for sc in range(NSC):
                ps3 = l3ps.tile([1, SC], F32, tag="l3", name=f"l3ps_{t}_{ic}_{sc}")
                nc.tensor.matmul(
                    ps3[:],
                    w3s[t][:, :, 0:1],
                    h2t[:, :, sc * SC : (sc + 1) * SC],
                    start=True,
                    stop=True,
                    perf_mode=DR,
                )
                ot = opool.tile([1, SC], F32, tag="o", name=f"o_{t}_{ic}_{sc}")
                nc.scalar.activation(
                    ot[:], ps3[:], AFT.Sigmoid, bias=b3s[t][:], scale=1.0 / (HS * WS)
                )
                nc.scalar.dma_start(out[t, :, n0 + sc * SC : n0 + (sc + 1) * SC], ot[:])


_NC_CACHE = {}


def _get_program(reps: int = 1, b2_nonzero: bool = False):
    key = (reps, b2_nonzero)
    if key not in _NC_CACHE:
        _NC_CACHE[key] = _build_program(reps, b2_nonzero)
    return _NC_CACHE[key]


def _prep_in_maps(x, W1, b1, W2, b2, W3, b3):
    f8 = mybir.dt.np(FP8)

    x = np.ascontiguousarray(np.asarray(x, dtype=np.float32))
    # [n, t, d] -> [t, d-in-block, d-block, n] (partition-major so the DMA's
    # source iteration (p, db, n) matches the SBUF destination), relu'd on device
    xv = (x * XS).reshape(N, T, NDB, 128)
    xbig = np.ascontiguousarray(xv.transpose(1, 3, 2, 0)).astype(f8)  # [16,128,8,8192]

    w1f = np.asarray(W1, np.float32) * WS
    # d = kb*256 + i*128 + p  ->  [T, p, kb, i, H1] (partition-major)
    w1r = np.ascontiguousarray(
        w1f.reshape(T, NKB1, 2, 128, H1).transpose(0, 3, 1, 2, 4)
    ).astype(f8)
    b1r = np.ascontiguousarray(
        (np.asarray(b1, np.float32) * HS).reshape(T, NHB, 128).transpose(0, 2, 1)
    )

    w2f = np.asarray(W2, np.float32) * WS
    w2r = np.ascontiguousarray(
        w2f.reshape(T, NPB2, 2, 128, H2).transpose(0, 3, 1, 2, 4)
    ).astype(f8)
    b2r = np.ascontiguousarray(
        (np.asarray(b2, np.float32) * HS).reshape(T, NKB2, 128).transpose(0, 2, 1)
    )

    w3f = np.asarray(W3, np.float32) * WS
    w3r = np.ascontiguousarray(w3f.reshape(T, 2, 128, 1).transpose(0, 2, 1, 3)).astype(f8)
    b3r = np.ascontiguousarray(np.asarray(b3, np.float32)).reshape(T, 1, 1)

    in_maps = []
    for c in range(NCORES):
        t0, t1 = TL * c, TL * (c + 1)
        in_maps.append(
            {
                "xq": xbig[t0:t1],
                "w1": w1r[t0:t1],
                "b1": b1r[t0:t1],
                "w2": w2r[t0:t1],
                "b2": b2r[t0:t1],
                "w3": w3r[t0:t1],
                "b3": b3r[t0:t1],
            }
        )

    return in_maps


def kernel(x, W1, b1, W2, b2, W3, b3):
    global LAST_RESULT
    bias_nonzero = bool(np.any(np.asarray(b1))) or bool(np.any(np.asarray(b2)))
    nc = _get_program(1, bias_nonzero)
    in_maps = _prep_in_maps(x, W1, b1, W2, b2, W3, b3)
    res = run_bass_kernel_spmd(nc, in_maps, core_ids=list(range(NCORES)), trace=TRACE)
    LAST_RESULT = res
    outs = np.stack([res.results[c]["out"] for c in range(NCORES)])  # [8, 2, 1, 8192]
    return np.ascontiguousarray(
        outs.reshape(T, N).T.reshape(B, S, T).astype(np.float32)
    )


def timed_run(inputs, reps, n_meas=3):
    """Per-iteration device time via an in-NEFF hardware loop of `reps`
    iterations vs 1: (t_reps - t_1) / (reps - 1). Isolates device exec
    from host prep + axon transfer (identical on both dispatches)."""
    import time as _time

    in_maps = _prep_in_maps(**inputs)
    bias_nonzero = bool(np.any(np.asarray(inputs["b1"]))) or bool(
        np.any(np.asarray(inputs["b2"]))
    )
    nc1 = _get_program(1, bias_nonzero)
    ncR = _get_program(reps, bias_nonzero)

    def _one(nc):
        t0 = _time.perf_counter()
        run_bass_kernel_spmd(nc, in_maps, core_ids=list(range(NCORES)))
        return _time.perf_counter() - t0

    _one(nc1)  # warm compile+cache
    _one(ncR)
    t1s, tRs = [], []
    for _ in range(n_meas):  # interleave to cancel drift
        t1s.append(_one(nc1))
        tRs.append(_one(ncR))
    deltas = sorted(tR - t1 for t1, tR in zip(t1s, tRs))
    med = deltas[len(deltas) // 2]
    per_iter_ns = med / (reps - 1) * 1e9
    return per_iter_ns, t1s, tRs


# revision 24
# speedup vs baseline: 3.8852x; 3.8852x over previous
"""Trainium2 Bass kernel for nn_LLMBinaryMultitaskMLPGenerator.

out[b,s,t] = sigmoid(relu(relu(relu(x) @ W1[t] + b1[t]) @ W2[t] + b2[t]) @ W3[t] + b3[t])

Sharding: task-parallel across 8 cores (2 tasks per core, all 8192 rows).

All three GEMMs run in fp8 (e4m3) with DoubleRow perf mode: operands are
packed [128, 2, ...] so each matmul contracts 256 deep at 2x the bf16
MAC rate, accumulating fp32 in PSUM. Host-side layout prep scales the
operands into e4m3's normal range (x*16, W*64, activations*8); PSUM
eviction folds the dequant scale + bias + relu/sigmoid into one
scalar-engine activation op (L1, L3) or a DVE tensor_scalar mult+max
(L2, valid because b2==0; a scalar-engine variant is built if any b2
is nonzero). relu(x) runs in place, split DVE / Pool engines.

Numerics (numpy sim vs fp32 reference): rel_l2 ~1e-2, gate is 2e-2.

Engine budget per 2048-col chunk (8 chunks/rep), HW estimates:
  PE ~18us, ACT ~12us, DVE ~8us, Pool ~5us, DMA ~6us in + tiny out.

Layouts (per core, TL=2 tasks):
  xq  [TL, 128, 8, N] fp8   = e4m3(16*x), d = db*128+p (partition-major)
  w1  [TL, 4, 128, 2, 512]  = e4m3(64*W1), d = kb*256 + i*128 + p
  b1  [TL, 4, 128, 1] f32   = 8*b1
  w2  [TL, 2, 128, 2, 256]  = e4m3(64*W2), h = pb*256 + i*128 + p
  b2  [TL, 2, 128, 1] f32   = 8*b2
  w3  [TL, 128, 2, 1] fp8   = e4m3(64*W3), k = i*128 + p
  b3  [TL, 1, 1] f32
  out [TL, 1, N] f32
"""

import sys

sys.path.insert(0, "/opt/trn_rl_repo")

from contextlib import ExitStack

import numpy as np

import concourse.bass as bass  # noqa: F401  (engine namespaces live on nc)
import concourse.mybir as mybir
import concourse.tile as tile
from concourse import bacc
from concourse.bass_utils import run_bass_kernel_spmd

import jax

jax.config.update("jax_compilation_cache_dir", "/tmp/jaxcache")
jax.config.update("jax_persistent_cache_min_compile_time_secs", 0.0)
jax.config.update("jax_persistent_cache_min_entry_size_bytes", -1)

F32 = mybir.dt.float32
BF16 = mybir.dt.bfloat16
FP8 = mybir.dt.float8e4
AFT = mybir.ActivationFunctionType
ALU = mybir.AluOpType
DR = mybir.MatmulPerfMode.DoubleRow

NCORES = 8
B, S, T, D, H1, H2 = 4, 2048, 16, 1024, 512, 256
N = B * S  # 8192 rows per task, replicated on every core
TL = T // NCORES  # 2 tasks per core
NDB = D // 128  # 8 x d-blocks
NKB1 = D // 256  # 4 L1 contraction pair-blocks
NHB = H1 // 128  # 4 h1 feature blocks
NPB2 = H1 // 256  # 2 L2 contraction pair-blocks
NKB2 = H2 // 128  # 2 h2 feature blocks
IC_CHUNK = 2048  # n-columns per pipeline chunk
SC = 512  # matmul moving free dim / PSUM bank width (fp32 max)
NIC = N // IC_CHUNK
NSC = IC_CHUNK // SC
NDVE = 5  # x-relu d-blocks on DVE; the rest go to Pool (gpsimd)
EVICT_L1_DVE = 6  # L1 psum groups per chunk evicted on DVE when biases are 0

XS, WS, HS = 16.0, 64.0, 8.0  # host-side quant scales for x, weights, activations

TRACE = False
ABLATE = set()  # timing-ablation flags: 'norelu', 'nol1', 'nol2', 'nol3'
LAST_RESULT = None  # BassKernelResults of the last kernel() call


def _build_program(reps: int = 1, b2_nonzero: bool = False):
    nc = bacc.Bacc("TRN2", target_bir_lowering=False, debug=False, num_devices=NCORES)

    xq = nc.dram_tensor("xq", [TL, 128, NDB, N], BF16, kind="ExternalInput").ap()
    w1 = nc.dram_tensor("w1", [TL, 128, NKB1, 2, H1], FP8, kind="ExternalInput").ap()
    b1 = nc.dram_tensor("b1", [TL, 128, NHB], F32, kind="ExternalInput").ap()
    w2 = nc.dram_tensor("w2", [TL, 128, NPB2, 2, H2], FP8, kind="ExternalInput").ap()
    b2 = nc.dram_tensor("b2", [TL, 128, NKB2], F32, kind="ExternalInput").ap()
    w3 = nc.dram_tensor("w3", [TL, 128, 2, 32], FP8, kind="ExternalInput").ap()
    b3 = nc.dram_tensor("b3", [TL, 128, 1], F32, kind="ExternalInput").ap()
    out = nc.dram_tensor("out", [TL, 1, N], F32, kind="ExternalOutput").ap()

    with tile.TileContext(nc) as tc, ExitStack() as ctx:
        wpool = ctx.enter_context(tc.tile_pool(name="w", bufs=1))
        xbpool = ctx.enter_context(tc.tile_pool(name="xb", bufs=2))
        xpool = ctx.enter_context(tc.tile_pool(name="x", bufs=3))
        h1pool = ctx.enter_context(tc.tile_pool(name="h1", bufs=4))
        h2pool = ctx.enter_context(tc.tile_pool(name="h2", bufs=2))
        opool = ctx.enter_context(tc.tile_pool(name="o", bufs=4))
        l1ps = ctx.enter_context(tc.tile_pool(name="l1ps", bufs=4, space="PSUM"))
        l2ps = ctx.enter_context(tc.tile_pool(name="l2ps", bufs=2, space="PSUM"))
        l3ps = ctx.enter_context(tc.tile_pool(name="l3ps", bufs=2, space="PSUM"))

        # --- persistent per-task weights/biases in SBUF (one DMA each) ---
        w1s, w2s, w3s, b1s, b2s, b3s = [], [], [], [], [], []
        for t in range(TL):
            w1s.append(wpool.tile([128, NKB1, 2, H1], FP8, tag=f"w1_{t}", name=f"w1t_{t}"))
            w2s.append(wpool.tile([128, NPB2, 2, H2], FP8, tag=f"w2_{t}", name=f"w2t_{t}"))
            # w3 column replicated 32x: each plain-fp8 L3 matmul then
            # writes a full 32-row psum block, so the packed [128,SC] psum
            # tile is fully initialized for its single sigmoid eviction
            w3s.append(wpool.tile([128, 2, 32], FP8, tag=f"w3_{t}", name=f"w3t_{t}"))
            b1s.append(wpool.tile([128, NHB], F32, tag=f"b1_{t}", name=f"b1t_{t}"))
            b2s.append(wpool.tile([128, NKB2], F32, tag=f"b2_{t}", name=f"b2t_{t}"))
            b3s.append(wpool.tile([128, 1], F32, tag=f"b3_{t}", name=f"b3t_{t}"))

        def _load_weights(t):
            nc.sync.dma_start(w1s[t][:], w1[t])
            nc.sync.dma_start(b1s[t][:], b1[t])
            nc.sync.dma_start(w2s[t][:], w2[t])
            nc.sync.dma_start(b2s[t][:], b2[t])
            nc.sync.dma_start(w3s[t][:], w3[t])
            nc.sync.dma_start(b3s[t][:], b3[t])

        def _body(load_weights):
            _pipeline(nc, tc, xq, out, w1s, w2s, w3s, b1s, b2s, b3s,
                      xbpool, xpool, h1pool, h2pool, opool, l1ps, l2ps, l3ps,
                      b2_nonzero, load_weights)

        if reps == 1:
            # weight DMAs are interleaved after the first x-chunk prefetches
            # so the first compute chunk starts as early as possible
            _body(_load_weights)
        else:
            for t in range(TL):
                _load_weights(t)
            with tc.For_i(0, reps, 1):
                _body(None)

    nc.compile()
    return nc


def _pipeline(nc, tc, xq, out, w1s, w2s, w3s, b1s, b2s, b3s,
              xbpool, xpool, h1pool, h2pool, opool, l1ps, l2ps, l3ps, b2_nonzero,
              load_weights=None):
    chunks = [(t, ic) for t in range(TL) for ic in range(NIC)]
    xtiles = {}

    def _prep(ci):
        # x: 4 batched DMAs per chunk, relu'd in place on DVE per kb-pair so
        # L1's kb-ordered matmuls can start as soon as pair 0 is ready.
        # Issued 2 chunks ahead of compute so the relu sits ahead of the
        # previous chunk's L2 evictions in the DVE queue.
        t, ic = chunks[ci]
        n0 = ic * IC_CHUNK
        # x arrives bf16 (DVE's fp8-input path is ~10x slow on HW; bf16 input
        # with fp8 cast-on-write runs at full rate), relu'd into an fp8 tile
        xb = xbpool.tile([128, NDB, IC_CHUNK], BF16, tag="xb", name=f"xb_{t}_{ic}")
        xs = xpool.tile([128, NDB, IC_CHUNK], FP8, tag="x", name=f"x_{t}_{ic}")
        for kb in range(NKB1):
            bpair = xb[:, 2 * kb : 2 * kb + 2, :]
            nc.sync.dma_start(
                bpair, xq[t, :, 2 * kb : 2 * kb + 2, n0 : n0 + IC_CHUNK]
            )
            if "norelu" not in ABLATE:
                nc.vector.tensor_scalar_max(
                    xs[:, 2 * kb : 2 * kb + 2, :], bpair, 0.0
                )
        xtiles[ci] = xs

    _prep(0)
    if load_weights is not None:
        load_weights(0)
    _prep(1)
    if load_weights is not None:
        for t in range(1, TL):
            load_weights(t)
    for ci, (t, ic) in enumerate(chunks):
            n0 = ic * IC_CHUNK
            xs = xtiles.pop(ci)
            if ci + 2 < len(chunks):
                _prep(ci + 2)

            # L1: h1 = 8*relu(z1 + b1), fp8, pair-packed over hb (scalar engine)
            h1t = [h1pool.tile([128, 2, IC_CHUNK], FP8, tag="h1", name=f"h1_{t}_{ic}_{pb}")
                   for pb in range(NPB2)]
            for hb in range(NHB if "nol1" not in ABLATE else 0):
                for sc in range(NSC):
                    ps = l1ps.tile([128, SC], F32, tag="l1", name=f"l1ps_{t}_{ic}_{hb}_{sc}")
                    for kb in range(NKB1):
                        nc.tensor.matmul(
                            ps[:],
                            w1s[t][:, kb, :, hb * 128 : (hb + 1) * 128],
                            xs[:, 2 * kb : 2 * kb + 2, sc * SC : (sc + 1) * SC],
                            start=(kb == 0),
                            stop=(kb == NKB1 - 1),
                            perf_mode=DR,
                        )
                    h1view = h1t[hb // 2][:, hb % 2, sc * SC : (sc + 1) * SC]
                    gidx = hb * NSC + sc
                    if not b2_nonzero and gidx >= NHB * NSC - EVICT_L1_DVE:
                        nc.vector.tensor_scalar(
                            h1view, ps[:], HS / (XS * WS), 0.0,
                            op0=ALU.mult, op1=ALU.max,
                        )
                    else:
                        nc.scalar.activation(
                            h1view, ps[:], AFT.Relu,
                            bias=b1s[t][:, hb : hb + 1],
                            scale=HS / (XS * WS),
                        )

            # L2: h2 = 8*relu(z2 + b2), fp8, pair-packed over kb (DVE eviction)
            h2t = h2pool.tile([128, 2, IC_CHUNK], FP8, tag="h2", name=f"h2_{t}_{ic}")
            for kb in range(NKB2 if ("nol2" not in ABLATE and "nol1" not in ABLATE) else 0):
                for sc in range(NSC):
                    ps = l2ps.tile([128, SC], F32, tag="l2", name=f"l2ps_{t}_{ic}_{kb}_{sc}")
                    for pb in range(NPB2):
                        nc.tensor.matmul(
                            ps[:],
                            w2s[t][:, pb, :, kb * 128 : (kb + 1) * 128],
                            h1t[pb][:, :, sc * SC : (sc + 1) * SC],
                            start=(pb == 0),
                            stop=(pb == NPB2 - 1),
                            perf_mode=DR,
                        )
                    h2view = h2t[:, kb, sc * SC : (sc + 1) * SC]
                    if b2_nonzero:
                        nc.scalar.activation(
                            h2view, ps[:], AFT.Relu,
                            bias=b2s[t][:, kb : kb + 1], scale=HS / (HS * WS),
                        )
                    else:
                        nc.vector.tensor_scalar(
                            h2view, ps[:], HS / (HS * WS), 0.0,
                            op0=ALU.mult, op1=ALU.max,
                        )

            # L3: the chunk's NSC [1,SC] logit rows are packed at partitions
            # 32*sc of ONE psum bank (plain-fp8 matmuls; dual-fp8 requires
            # dst partition 0), evicted by a single sigmoid activation and a
            # single partition-strided out DMA — 1 ACT instr/chunk instead
            # of NSC.
            if "nol3" not in ABLATE and "nol2" not in ABLATE and "nol1" not in ABLATE:
                ps3 = l3ps.tile([128, SC], F32, tag="l3", name=f"l3ps_{t}_{ic}")
                for sc in range(NSC):
                    for kb in range(NKB2):
                        nc.tensor.matmul(
                            ps3[32 * sc : 32 * sc + 32, :],
                            w3s[t][:, kb, :],
                            h2t[:, kb, sc * SC : (sc + 1) * SC],
                            start=(kb == 0),
                            stop=(kb == NKB2 - 1),
                            perf_mode=None,
                            tile_position=(0, 32 * sc),
                        )
                ot = opool.tile([128, SC], F32, tag="o", name=f"o_{t}_{ic}")
                nc.scalar.activation(
                    ot[:], ps3[:], AFT.Sigmoid, bias=b3s[t][:], scale=1.0 / (HS * WS)
                )
                nc.scalar.dma_start(
                    out[t, :, n0 : n0 + IC_CHUNK], ot[0 : 32 * NSC : 32, :]
                )


_NC_CACHE = {}


def _get_program(reps: int = 1, b2_nonzero: bool = False):
    key = (reps, b2_nonzero)
    if key not in _NC_CACHE:
        _NC_CACHE[key] = _build_program(reps, b2_nonzero)
    return _NC_CACHE[key]


def _prep_in_maps(x, W1, b1, W2, b2, W3, b3):
    f8 = mybir.dt.np(FP8)

    x = np.ascontiguousarray(np.asarray(x, dtype=np.float32))
    # [n, t, d] -> [t, d-in-block, d-block, n] (partition-major so the DMA's
    # source iteration (p, db, n) matches the SBUF destination), relu'd on device
    bf16 = mybir.dt.np(BF16)
    xv = (x * XS).reshape(N, T, NDB, 128)
    xbig = np.ascontiguousarray(xv.transpose(1, 3, 2, 0)).astype(bf16)  # [16,128,8,8192]

    w1f = np.asarray(W1, np.float32) * WS
    # d = kb*256 + i*128 + p  ->  [T, p, kb, i, H1] (partition-major)
    w1r = np.ascontiguousarray(
        w1f.reshape(T, NKB1, 2, 128, H1).transpose(0, 3, 1, 2, 4)
    ).astype(f8)
    b1r = np.ascontiguousarray(
        (np.asarray(b1, np.float32) * HS).reshape(T, NHB, 128).transpose(0, 2, 1)
    )

    w2f = np.asarray(W2, np.float32) * WS
    w2r = np.ascontiguousarray(
        w2f.reshape(T, NPB2, 2, 128, H2).transpose(0, 3, 1, 2, 4)
    ).astype(f8)
    b2r = np.ascontiguousarray(
        (np.asarray(b2, np.float32) * HS).reshape(T, NKB2, 128).transpose(0, 2, 1)
    )

    w3f = np.asarray(W3, np.float32) * WS
    w3r = np.ascontiguousarray(
        np.broadcast_to(
            w3f.reshape(T, 2, 128, 1).transpose(0, 2, 1, 3), (T, 128, 2, 32)
        )
    ).astype(f8)
    b3r = np.ascontiguousarray(
        np.broadcast_to(np.asarray(b3, np.float32).reshape(T, 1, 1), (T, 128, 1))
    )

    in_maps = []
    for c in range(NCORES):
        t0, t1 = TL * c, TL * (c + 1)
        in_maps.append(
            {
                "xq": xbig[t0:t1],
                "w1": w1r[t0:t1],
                "b1": b1r[t0:t1],
                "w2": w2r[t0:t1],
                "b2": b2r[t0:t1],
                "w3": w3r[t0:t1],
                "b3": b3r[t0:t1],
            }
        )

    return in_maps


def kernel(x, W1, b1, W2, b2, W3, b3):
    global LAST_RESULT
    bias_nonzero = bool(np.any(np.asarray(b1))) or bool(np.any(np.asarray(b2)))
    nc = _get_program(1, bias_nonzero)
    in_maps = _prep_in_maps(x, W1, b1, W2, b2, W3, b3)
    res = run_bass_kernel_spmd(nc, in_maps, core_ids=list(range(NCORES)), trace=TRACE)
    LAST_RESULT = res
    outs = np.stack([res.results[c]["out"] for c in range(NCORES)])  # [8, 2, 1, 8192]
    return np.ascontiguousarray(
        outs.reshape(T, N).T.reshape(B, S, T).astype(np.float32)
    )


def timed_run(inputs, reps, n_meas=3):
    """Per-iteration device time via an in-NEFF hardware loop of `reps`
    iterations vs 1: (t_reps - t_1) / (reps - 1). Isolates device exec
    from host prep + axon transfer (identical on both dispatches)."""
    import time as _time

    in_maps = _prep_in_maps(**inputs)
    bias_nonzero = bool(np.any(np.asarray(inputs["b1"]))) or bool(
        np.any(np.asarray(inputs["b2"]))
    )
    nc1 = _get_program(1, bias_nonzero)
    ncR = _get_program(reps, bias_nonzero)

    def _one(nc):
        t0 = _time.perf_counter()
        run_bass_kernel_spmd(nc, in_maps, core_ids=list(range(NCORES)))
        return _time.perf_counter() - t0

    _one(nc1)  # warm compile+cache
    _one(ncR)
    t1s, tRs = [], []
    for _ in range(n_meas):  # interleave to cancel drift
        t1s.append(_one(nc1))
        tRs.append(_one(ncR))
    deltas = sorted(tR - t1 for t1, tR in zip(t1s, tRs))
    med = deltas[len(deltas) // 2]
    per_iter_ns = med / (reps - 1) * 1e9
    return per_iter_ns, t1s, tRs
